# revision 32
# baseline (speedup 1.0000x reference)
"""DeepFM (nn_DeepFM_77558519431939) Trainium2 Bass kernel.

Strategy (8 NeuronCores, SPMD, no collectives):
  - Replicate the embedding table on every core; data-parallel the batch
    (16384 samples -> 2048 per core).  Each gathered row is fetched exactly
    once across the fleet, and there is no all-to-all.
  - Host-side prep builds an augmented table [S, 12]: 10 embedding dims,
    w_first value (first-order weight) in col 10, zero pad in col 11.
  - The gather bottleneck is the GpSimd SWDGE issue path: the HW supports
    exactly ONE offset per partition per indirect DMA (~1.07us engine +
    ~0.31us dispatch apiece, engine-serial), i.e. 128 rows per instruction.
    Instruction count is minimized by fusing the two smallest fields into a
    host-built cross-product table t67[v6*1000+v7] = [row6 ; row7] so one
    24-element descriptor fetches both rows: 7 gathers per 128-sample block
    instead of 8 (112 instructions total).  All gathers are issued up front
    so the entire compute pipeline hides under the SWDGE stream.
  - Gathered rows land sample-on-partition; PE transposes flip them into a
    feature-major activation matrix X [104, 2048]:
        rows f*12+e (e<10): emb dim e of field f
        rows f*12+10:       w_first value of field f
        rows f*12+11:       zero pad
        rows 96..102:       raw dense features (transposed on host)
        row 103:            constant 1.0 (bias row)
  - Compute runs on 512-column tiles with float32r matmuls (1 cycle/row at
    >=256 moving dim vs 4 for fp32), with all the small weights folded on
    the host:
        H1 = relu(W1s^T X)            (dense-proj + b1 folded into W1s)
        H2 = relu(W2^T H1 + b2)
        SD = SDW^T X                  (cols 0..9 = s, 10..19 = dense_emb)
        XSQ = X[0:96]^2, SD2 = SD^2
        FIN = a1^T X + esq^T XSQ + es2^T SD2 + W3^T H2
        out = sigmoid(FIN)
"""

import os
from contextlib import ExitStack

import numpy as np

import concourse.bass as bass
import concourse.bacc as bacc
import concourse.mybir as mybir
import concourse.tile as tile

# ---- problem constants (hardcoded; must match the reference) ----
VOCABS = [1000000, 500000, 200000, 100000, 50000, 10000, 5000, 1000]
S = int(np.sum(VOCABS))  # 1,866,000
OFFSETS = np.concatenate([[0], np.cumsum(VOCABS)[:-1]]).astype(np.int64)
B = 16384
EMB = 10
N_DENSE = 7
F = len(VOCABS)  # 8
HID = 128

N_CORES = 8
BL = B // N_CORES  # 2048 per core
RW = 12            # augmented table row width (10 emb + wf + pad)
KX = 104           # X partition rows: 96 gathered + 7 dense + 1 const
NBLK = BL // 128   # 16 sample blocks of 128
NT = BL // 512     # 4 column tiles of 512
BPT = 4            # blocks per tile
CW = BPT * F * RW  # gather chunk width per partition (384 f32)
NSQ = 116          # unused (kept for reference): 96 emb^2 + 10 s^2 + 10 demb^2
WPK = 409          # packed weight tensor width

F32 = mybir.dt.float32
F32R = mybir.dt.float32r
I32 = mybir.dt.int32

_cached = {}


def _build_program(debug_dump=False):
    """Build the SPMD Bass program (same for all cores)."""
    nc = bacc.Bacc("TRN2", target_bir_lowering=False, debug=False)

    tab_d = nc.dram_tensor("tab", [S, RW], F32, kind="ExternalInput").ap()
    # cross-product table for the two smallest fields (vocab 5000 x 1000):
    # row v6*1000+v7 = [field6 row ; field7 row], so ONE 24-element
    # descriptor fetches both rows (gather instructions are the bottleneck:
    # 128 descriptors each, ~1.04us of serial GpSimd time apiece)
    t67_d = nc.dram_tensor("t67", [VOCABS[6] * VOCABS[7], 2 * RW], F32,
                           kind="ExternalInput").ap()
    idx_d = nc.dram_tensor("idxs", [128, NBLK * 7], I32, kind="ExternalInput").ap()
    dn8_d = nc.dram_tensor("dn8", [8, BL], F32, kind="ExternalInput").ap()
    # all small weights packed into one tensor: one DMA, one sem wait
    # cols: idn 0:128 | w1s 128:256 | w2 256:384 | b2 384 | sdw 385:406 |
    #       esqc 406 | w3 407
    wpk_d = nc.dram_tensor("wpk", [128, WPK], F32, kind="ExternalInput").ap()
    out_d = nc.dram_tensor("out", [1, BL], F32, kind="ExternalOutput").ap()
    if debug_dump:
        xdmp_d = nc.dram_tensor("xdmp", [KX, BL], F32, kind="ExternalOutput").ap()
        fdmp_d = nc.dram_tensor("fdmp", [1, BL], F32, kind="ExternalOutput").ap()
        gdmp_d = nc.dram_tensor("gdmp", [128, NT * CW], F32,
                                kind="ExternalOutput").ap()

    with ExitStack() as ctx:
        tc = ctx.enter_context(tile.TileContext(nc))
        const = ctx.enter_context(tc.tile_pool(name="const", bufs=1))
        gpool = ctx.enter_context(tc.tile_pool(name="g", bufs=NT))
        hpool = ctx.enter_context(tc.tile_pool(name="h", bufs=2))
        qpool = ctx.enter_context(tc.tile_pool(name="q", bufs=2))
        pp_x = ctx.enter_context(tc.tile_pool(name="ppx", bufs=2, space="PSUM"))
        pp_h = ctx.enter_context(tc.tile_pool(name="pph", bufs=2, space="PSUM"))
        pp_s = ctx.enter_context(tc.tile_pool(name="pps", bufs=2, space="PSUM"))
        pp_f = ctx.enter_context(tc.tile_pool(name="ppf", bufs=2, space="PSUM"))

        # index tile first: the gathers depend only on it (chunk-0 columns
        # land in a separate small DMA so the first gathers start sooner)
        idx_t = const.tile([128, NBLK * 7], I32)
        nc.sync.dma_start(idx_t[:, 0:BPT * 7], idx_d[:, 0:BPT * 7])
        nc.sync.dma_start(idx_t[:, BPT * 7:], idx_d[:, BPT * 7:])

        # constants: one packed tile, sliced below
        wpk_t = const.tile([128, WPK], F32)
        nc.sync.dma_start(wpk_t[:], wpk_d[:])
        idn_t = wpk_t[:, 0:128]
        b2_t = wpk_t[:, 384:385]

        # f32r copy of the matmul weights (the BIR verifier requires every
        # producer of an FP32r matmul operand to round-on-write)
        wpkr_t = const.tile([128, 281], F32R)
        nc.vector.tensor_copy(wpkr_t[:], wpk_t[:, 128:409])
        w1s_t = wpkr_t[0:KX, 0:128]
        w2_t = wpkr_t[:, 128:256]
        sdw_t = wpkr_t[0:KX, 257:277]
        a1_t = wpkr_t[0:KX, 277:278]
        esq_t = wpkr_t[0:96, 278:279]
        w3_t = wpkr_t[:, 279:280]
        es2_t = wpkr_t[0:20, 280:281]

        # X: feature-major activations (f32r: consumed by f32r matmuls)
        x_t = const.tile([KX, BL], F32R)
        dn8_t = const.tile([8, BL], F32)
        nc.sync.dma_start(dn8_t[:], dn8_d[:])
        nc.vector.tensor_copy(x_t[96:104, :], dn8_t[:])

        out_sb = const.tile([1, BL], F32)
        if debug_dump:
            fin_sb = const.tile([1, BL], F32)

        RELU = mybir.ActivationFunctionType.Relu
        SQUARE = mybir.ActivationFunctionType.Square
        SIGMOID = mybir.ActivationFunctionType.Sigmoid

        # issue ALL gathers up front so the whole compute pipeline hides
        # under the serial GpSimd SWDGE stream (the hard bottleneck: the HW
        # supports only one offset per partition per indirect DMA, ~1.04us
        # each, 128 instructions total)
        gbs = []
        for c in range(NT):
            gb = gpool.tile([128, CW], F32, tag="g")
            for j in range(BPT):
                for f in range(7):
                    col = (c * BPT + j) * 7 + f
                    base = (j * F + f) * RW
                    src = tab_d[:] if f < 6 else t67_d[:]
                    width = RW if f < 6 else 2 * RW
                    nc.gpsimd.indirect_dma_start(
                        out=gb[:, base:base + width],
                        out_offset=None,
                        in_=src,
                        in_offset=bass.IndirectOffsetOnAxis(
                            ap=idx_t[:, col:col + 1], axis=0
                        ),
                    )
            gbs.append(gb)

        # compute on 256-column tiles (2 blocks): still >=256 moving dim so
        # f32r matmuls stay at 1 cycle/row, but the post-last-gather tail is
        # only a 2-block chain instead of 4
        TW = 256
        for t in range(NT * 2):
            c, half = t // 2, t % 2
            cols = slice(TW * t, TW * (t + 1))
            gb = gbs[c]
            xp = pp_x.tile([96, TW], F32, tag="xp")
            for jj in range(2):
                j = half * 2 + jj
                nc.tensor.transpose(
                    out=xp[:, 128 * jj:128 * (jj + 1)],
                    in_=gb[:, 96 * j:96 * (j + 1)],
                    identity=idn_t,
                )
            nc.vector.tensor_copy(x_t[0:96, cols], xp[:])

            xr = x_t[0:KX, cols]

            # MLP
            h1p = pp_h.tile([HID, TW], F32, tag="hp")
            nc.tensor.matmul(out=h1p[:], lhsT=w1s_t, rhs=xr,
                             start=True, stop=True)
            h1_t = hpool.tile([HID, TW], F32R, tag="h")
            nc.scalar.activation(h1_t[:], h1p[:], RELU)
            h2p = pp_h.tile([HID, TW], F32, tag="hp")
            nc.tensor.matmul(out=h2p[:], lhsT=w2_t, rhs=h1_t[:],
                             start=True, stop=True)
            h2_t = hpool.tile([HID, TW], F32R, tag="h")
            nc.scalar.activation(h2_t[:], h2p[:], RELU, bias=b2_t)

            # s / dense_emb rows
            sdp = pp_s.tile([20, TW], F32, tag="sd")
            nc.tensor.matmul(out=sdp[:], lhsT=sdw_t, rhs=xr,
                             start=True, stop=True)

            # squares for the FM cross term (partition-aligned tiles)
            xsq = qpool.tile([96, TW], F32R, tag="xsq")
            nc.vector.tensor_mul(xsq[:], x_t[0:96, cols], x_t[0:96, cols])
            sd2 = qpool.tile([20, TW], F32R, tag="sd2")
            nc.scalar.activation(sd2[:], sdp[:], SQUARE)

            # final accumulation + sigmoid
            fin = pp_f.tile([1, TW], F32, tag="fin")
            nc.tensor.matmul(out=fin[:], lhsT=a1_t, rhs=xr,
                             start=True, stop=False)
            nc.tensor.matmul(out=fin[:], lhsT=esq_t, rhs=xsq[:],
                             start=False, stop=False)
            nc.tensor.matmul(out=fin[:], lhsT=es2_t, rhs=sd2[:],
                             start=False, stop=False)
            nc.tensor.matmul(out=fin[:], lhsT=w3_t, rhs=h2_t[:],
                             start=False, stop=True)
            if debug_dump:
                nc.vector.tensor_copy(fin_sb[:, cols], fin[:])
            nc.scalar.activation(out_sb[:, cols], fin[:], SIGMOID)
            if t == NT * 2 - 2:
                # fire the first 7/8 of the output while the last tile drains
                nc.sync.dma_start(out_d[:, 0:TW * (t + 1)],
                                  out_sb[:, 0:TW * (t + 1)])

        nc.sync.dma_start(out_d[:, TW * (NT * 2 - 1):], out_sb[:, TW * (NT * 2 - 1):])
        if debug_dump:
            nc.sync.dma_start(xdmp_d[:], x_t[:].bitcast(F32))
            nc.sync.dma_start(fdmp_d[:], fin_sb[:])
            for c in range(NT):
                nc.sync.dma_start(gdmp_d[:, c * CW:(c + 1) * CW], gbs[c][:])

    nc.compile()
    return nc


def _host_prep(sparse_feature, dense_feature, emb_table, W_dense, b_dense,
               w_first, b_first, W1, b1, W2, b2, W3, b3):
    """Build the augmented table, folded weights, and per-core in_maps."""
    f32 = np.float32
    emb_table = np.asarray(emb_table, dtype=f32)
    W_dense = np.asarray(W_dense, dtype=f32)      # [10, 7]
    b_dense = np.asarray(b_dense, dtype=f32)      # [10]
    w_first = np.asarray(w_first, dtype=f32)      # [S+7]
    b_first = np.asarray(b_first, dtype=f32)      # [1]
    W1 = np.asarray(W1, dtype=f32)                # [90, 128]
    b1 = np.asarray(b1, dtype=f32)                # [128]
    W2 = np.asarray(W2, dtype=f32)                # [128, 128]
    b2 = np.asarray(b2, dtype=f32)                # [128]
    W3 = np.asarray(W3, dtype=f32)                # [128, 1]
    b3 = np.asarray(b3, dtype=f32)                # [1]

    tab = np.zeros((S, RW), dtype=f32)
    tab[:, :EMB] = emb_table
    tab[:, EMB] = w_first[:S]

    w1s = np.zeros((KX, HID), dtype=f32)
    for f in range(F):
        w1s[f * RW:f * RW + EMB] = W1[f * EMB:(f + 1) * EMB]
    w1s[96:103] = W_dense.T @ W1[F * EMB:]               # [7,128]
    w1s[103] = b1 + b_dense @ W1[F * EMB:]

    # sdw: cols 0..9 -> s (sum of field embs + dense emb), 10..19 -> dense
    # emb alone
    sdw = np.zeros((KX, 20), dtype=f32)
    for f in range(F):
        for e in range(EMB):
            sdw[f * RW + e, e] = 1.0
    sdw[96:103, 0:10] = W_dense.T
    sdw[103, 0:10] = b_dense
    sdw[96:103, 10:20] = W_dense.T
    sdw[103, 10:20] = b_dense

    # a1: first-order linear term (wf rows + dense wfirst + biases)
    a1 = np.zeros((KX, 1), dtype=f32)
    for f in range(F):
        a1[f * RW + EMB] = 1.0
    a1[96:103, 0] = w_first[S:]
    a1[103] = b_first[0] + b3[0]

    # esq: -0.5 coefficients for the x^2 rows; es2: [+0.5 s^2, -0.5 demb^2]
    esq = np.zeros((96, 1), dtype=f32)
    for f in range(F):
        esq[f * RW:f * RW + EMB] = -0.5
    es2 = np.zeros((20, 1), dtype=f32)
    es2[0:10] = 0.5
    es2[10:20] = -0.5

    idx_g = (np.asarray(sparse_feature, dtype=np.int64)
             + OFFSETS[None, :]).astype(np.int32)         # [B, F]
    dense = np.asarray(dense_feature, dtype=f32)          # [B, 7]

    # cross-product table for fields 6 and 7
    t6 = tab[OFFSETS[6]:OFFSETS[6] + VOCABS[6]]           # [5000, 12]
    t7 = tab[OFFSETS[7]:OFFSETS[7] + VOCABS[7]]           # [1000, 12]
    t67 = np.empty((VOCABS[6] * VOCABS[7], 2 * RW), dtype=f32)
    t67[:, :RW] = np.repeat(t6, VOCABS[7], axis=0)
    t67[:, RW:] = np.tile(t7, (VOCABS[6], 1))
    sp = np.asarray(sparse_feature, dtype=np.int64)
    idx67 = (sp[:, 6] * VOCABS[7] + sp[:, 7]).astype(np.int32)  # [B]
    idx7 = np.concatenate([idx_g[:, :6], idx67[:, None]], axis=1)  # [B, 7]

    wpk = np.zeros((128, WPK), dtype=f32)
    wpk[:, 0:128] = np.eye(128, dtype=f32)
    wpk[0:KX, 128:256] = w1s
    wpk[:, 256:384] = W2
    wpk[:, 384] = b2
    wpk[0:KX, 385:405] = sdw
    wpk[0:KX, 405] = a1[:, 0]
    wpk[0:96, 406] = esq[:, 0]
    wpk[:, 407] = W3.reshape(HID)
    wpk[0:20, 408] = es2[:, 0]

    common = {"tab": tab, "t67": t67, "wpk": wpk}
    in_maps = []
    for c in range(N_CORES):
        lo, hi = c * BL, (c + 1) * BL
        lg = idx7[lo:hi].reshape(NBLK, 128, 7)
        idxs = np.ascontiguousarray(
            lg.transpose(1, 0, 2).reshape(128, NBLK * 7))  # [128, 112]
        dn8 = np.ones((8, BL), dtype=f32)
        dn8[:7] = dense[lo:hi].T
        in_maps.append(dict(common, idxs=idxs, dn8=dn8))
    return in_maps


def _get_program(debug_dump=False):
    key = ("nc", debug_dump)
    if key not in _cached:
        _cached[key] = _build_program(debug_dump)
    return _cached[key]


def run_on_device(in_maps, trace=False, debug_dump=False):
    """Run the SPMD program on 8 NeuronCores.  Returns (results, exec_time_ns)."""
    from concourse.bass_utils import run_bass_kernel_spmd

    nc = _get_program(debug_dump)
    res = run_bass_kernel_spmd(nc, in_maps, list(range(N_CORES)), trace=trace)
    return res.results, res.exec_time_ns


def kernel(**inputs):
    in_maps = _host_prep(**inputs)
    results, _ = run_on_device(in_maps, trace=False)
    out = np.concatenate([results[c]["out"].reshape(BL) for c in range(N_CORES)])
    return out.astype(np.float32)


# revision 35
# speedup vs baseline: 1.1721x; 1.1721x over previous
"""DeepFM (nn_DeepFM_77558519431939) Trainium2 Bass kernel.

Strategy (8 NeuronCores, SPMD, no collectives):
  - Replicate the embedding table on every core; data-parallel the batch
    (16384 samples -> 2048 per core).  Each gathered row is fetched exactly
    once across the fleet, and there is no all-to-all.
  - Host-side prep builds an augmented table [S, 12]: 10 embedding dims,
    w_first value (first-order weight) in col 10, zero pad in col 11.
  - The gather bottleneck is the GpSimd SWDGE issue path: the HW supports
    exactly ONE offset per partition per indirect DMA (~1.07us engine +
    ~0.31us dispatch apiece, engine-serial), i.e. 128 rows per instruction.
    Instruction count is minimized by fusing the two smallest fields into a
    host-built cross-product table t67[v6*1000+v7] = [row6 ; row7] so one
    24-element descriptor fetches both rows: 7 gathers per 128-sample block
    instead of 8 (112 instructions total).  All gathers are issued up front
    so the entire compute pipeline hides under the SWDGE stream.
  - Gathered rows land sample-on-partition; PE transposes flip them into a
    feature-major activation matrix X [104, 2048]:
        rows f*12+e (e<10): emb dim e of field f
        rows f*12+10:       w_first value of field f
        rows f*12+11:       zero pad
        rows 96..102:       raw dense features (transposed on host)
        row 103:            constant 1.0 (bias row)
  - Compute runs on 512-column tiles with float32r matmuls (1 cycle/row at
    >=256 moving dim vs 4 for fp32), with all the small weights folded on
    the host:
        H1 = relu(W1s^T X)            (dense-proj + b1 folded into W1s)
        H2 = relu(W2^T H1 + b2)
        SD = SDW^T X                  (cols 0..9 = s, 10..19 = dense_emb)
        XSQ = X[0:96]^2, SD2 = SD^2
        FIN = a1^T X + esq^T XSQ + es2^T SD2 + W3^T H2
        out = sigmoid(FIN)
"""

import os
from contextlib import ExitStack

import numpy as np

import concourse.bass as bass
import concourse.bacc as bacc
import concourse.mybir as mybir
import concourse.tile as tile

# ---- problem constants (hardcoded; must match the reference) ----
VOCABS = [1000000, 500000, 200000, 100000, 50000, 10000, 5000, 1000]
S = int(np.sum(VOCABS))  # 1,866,000
OFFSETS = np.concatenate([[0], np.cumsum(VOCABS)[:-1]]).astype(np.int64)
B = 16384
EMB = 10
N_DENSE = 7
F = len(VOCABS)  # 8
HID = 128

N_CORES = 8
BL = B // N_CORES  # 2048 per core
RW = 12            # augmented table row width (10 emb + wf + pad)
KX = 104           # X partition rows: 96 gathered + 7 dense + 1 const
NBLK = BL // 128   # 16 sample blocks of 128
NT = BL // 512     # 4 column tiles of 512
BPT = 4            # blocks per tile
CW = BPT * F * RW  # gather chunk width per partition (384 f32)
NSQ = 116          # unused (kept for reference): 96 emb^2 + 10 s^2 + 10 demb^2
WPK = 409          # packed weight tensor width

F32 = mybir.dt.float32
F32R = mybir.dt.float32r
I32 = mybir.dt.int32

_cached = {}


def _build_program(debug_dump=False):
    """Build the SPMD Bass program (same for all cores)."""
    nc = bacc.Bacc("TRN2", target_bir_lowering=False, debug=False)

    tab_d = nc.dram_tensor("tab", [S, RW], F32, kind="ExternalInput").ap()
    # cross-product table for the two smallest fields (vocab 5000 x 1000):
    # row v6*1000+v7 = [field6 row ; field7 row], so ONE 24-element
    # descriptor fetches both rows (gather instructions are the bottleneck:
    # 128 descriptors each, ~1.04us of serial GpSimd time apiece)
    t67_d = nc.dram_tensor("t67", [VOCABS[6] * VOCABS[7], 2 * RW], F32,
                           kind="ExternalInput").ap()
    idx_d = nc.dram_tensor("idxs", [128, NBLK * 7], I32, kind="ExternalInput").ap()
    dn8_d = nc.dram_tensor("dn8", [8, BL], F32, kind="ExternalInput").ap()
    # all small weights packed into one tensor: one DMA, one sem wait
    # cols: idn 0:128 | w1s 128:256 | w2 256:384 | b2 384 | sdw 385:406 |
    #       esqc 406 | w3 407
    wpk_d = nc.dram_tensor("wpk", [128, WPK], F32, kind="ExternalInput").ap()
    out_d = nc.dram_tensor("out", [1, BL], F32, kind="ExternalOutput").ap()
    if debug_dump:
        xdmp_d = nc.dram_tensor("xdmp", [KX, BL], F32, kind="ExternalOutput").ap()
        fdmp_d = nc.dram_tensor("fdmp", [1, BL], F32, kind="ExternalOutput").ap()
        gdmp_d = nc.dram_tensor("gdmp", [128, NT * CW], F32,
                                kind="ExternalOutput").ap()

    with ExitStack() as ctx:
        tc = ctx.enter_context(tile.TileContext(nc))
        const = ctx.enter_context(tc.tile_pool(name="const", bufs=1))
        gpool = ctx.enter_context(tc.tile_pool(name="g", bufs=NT))
        hpool = ctx.enter_context(tc.tile_pool(name="h", bufs=2))
        qpool = ctx.enter_context(tc.tile_pool(name="q", bufs=2))
        pp_x = ctx.enter_context(tc.tile_pool(name="ppx", bufs=2, space="PSUM"))
        pp_h = ctx.enter_context(tc.tile_pool(name="pph", bufs=2, space="PSUM"))
        pp_s = ctx.enter_context(tc.tile_pool(name="pps", bufs=2, space="PSUM"))
        pp_f = ctx.enter_context(tc.tile_pool(name="ppf", bufs=2, space="PSUM"))

        # index tile first: the gathers depend only on it (chunk-0 columns
        # in a separate small DMA so the first gathers start sooner)
        idx_t = const.tile([128, NBLK * 7], I32)
        nc.sync.dma_start(idx_t[:, 0:BPT * 7], idx_d[:, 0:BPT * 7])
        nc.sync.dma_start(idx_t[:, BPT * 7:], idx_d[:, BPT * 7:])

        # constants: one packed tile, sliced below
        wpk_t = const.tile([128, WPK], F32)
        nc.sync.dma_start(wpk_t[:], wpk_d[:])
        idn_t = wpk_t[:, 0:128]
        b2_t = wpk_t[:, 384:385]

        # f32r copy of the matmul weights (the BIR verifier requires every
        # producer of an FP32r matmul operand to round-on-write)
        wpkr_t = const.tile([128, 281], F32R)
        nc.vector.tensor_copy(wpkr_t[:], wpk_t[:, 128:409])
        w1s_t = wpkr_t[0:KX, 0:128]
        w2_t = wpkr_t[:, 128:256]
        sdw_t = wpkr_t[0:KX, 257:277]
        a1_t = wpkr_t[0:KX, 277:278]
        esq_t = wpkr_t[0:96, 278:279]
        w3_t = wpkr_t[:, 279:280]
        es2_t = wpkr_t[0:20, 280:281]

        # X: feature-major activations (f32r: consumed by f32r matmuls)
        x_t = const.tile([KX, BL], F32R)
        dn8_t = const.tile([8, BL], F32)
        nc.sync.dma_start(dn8_t[:], dn8_d[:])
        nc.vector.tensor_copy(x_t[96:104, :], dn8_t[:])

        out_sb = const.tile([1, BL], F32)
        if debug_dump:
            fin_sb = const.tile([1, BL], F32)

        RELU = mybir.ActivationFunctionType.Relu
        SQUARE = mybir.ActivationFunctionType.Square
        SIGMOID = mybir.ActivationFunctionType.Sigmoid

        # issue ALL gathers up front so the whole compute pipeline hides
        # under the serial GpSimd SWDGE stream (the hard bottleneck: the HW
        # supports only one offset per partition per indirect DMA, ~1.04us
        # each, 128 instructions total)
        gbs = []
        for c in range(NT):
            gb = gpool.tile([128, CW], F32, tag="g")
            for j in range(BPT):
                for f in range(7):
                    col = (c * BPT + j) * 7 + f
                    base = (j * F + f) * RW
                    src = tab_d[:] if f < 6 else t67_d[:]
                    width = RW if f < 6 else 2 * RW
                    nc.gpsimd.indirect_dma_start(
                        out=gb[:, base:base + width],
                        out_offset=None,
                        in_=src,
                        in_offset=bass.IndirectOffsetOnAxis(
                            ap=idx_t[:, col:col + 1], axis=0
                        ),
                    )
            gbs.append(gb)

        # chunks 0-2 compute as one 512-col tile; the LAST chunk is split
        # into two 256-col halves so the serial tail after the final gather
        # is only a 2-block chain (f32r matmuls stay 1 cyc/row at >=256)
        for c in range(NT):
            subs = [(0, 512)] if c < NT - 1 else [(0, 256), (256, 256)]
            gb = gbs[c]
            for off, w in subs:
                cols = slice(512 * c + off, 512 * c + off + w)
                xp = pp_x.tile([96, 512], F32, tag="xp")
                for jj in range(w // 128):
                    j = off // 128 + jj
                    nc.tensor.transpose(
                        out=xp[:, 128 * jj:128 * (jj + 1)],
                        in_=gb[:, 96 * j:96 * (j + 1)],
                        identity=idn_t,
                    )
                nc.vector.tensor_copy(x_t[0:96, cols], xp[:, 0:w])

                xr = x_t[0:KX, cols]

                # MLP
                h1p = pp_h.tile([HID, 512], F32, tag="hp")
                nc.tensor.matmul(out=h1p[:, 0:w], lhsT=w1s_t, rhs=xr,
                                 start=True, stop=True)
                h1_t = hpool.tile([HID, 512], F32R, tag="h")
                nc.scalar.activation(h1_t[:, 0:w], h1p[:, 0:w], RELU)
                h2p = pp_h.tile([HID, 512], F32, tag="hp")
                nc.tensor.matmul(out=h2p[:, 0:w], lhsT=w2_t, rhs=h1_t[:, 0:w],
                                 start=True, stop=True)
                h2_t = hpool.tile([HID, 512], F32R, tag="h")
                nc.scalar.activation(h2_t[:, 0:w], h2p[:, 0:w], RELU, bias=b2_t)

                # s / dense_emb rows
                sdp = pp_s.tile([20, 512], F32, tag="sd")
                nc.tensor.matmul(out=sdp[:, 0:w], lhsT=sdw_t, rhs=xr,
                                 start=True, stop=True)

                # squares for the FM cross term (partition-aligned tiles)
                xsq = qpool.tile([96, 512], F32R, tag="xsq")
                nc.vector.tensor_mul(xsq[:, 0:w], x_t[0:96, cols],
                                     x_t[0:96, cols])
                sd2 = qpool.tile([20, 512], F32R, tag="sd2")
                nc.scalar.activation(sd2[:, 0:w], sdp[:, 0:w], SQUARE)

                # final accumulation + sigmoid
                fin = pp_f.tile([1, 512], F32, tag="fin")
                nc.tensor.matmul(out=fin[:, 0:w], lhsT=a1_t, rhs=xr,
                                 start=True, stop=False)
                nc.tensor.matmul(out=fin[:, 0:w], lhsT=esq_t, rhs=xsq[:, 0:w],
                                 start=False, stop=False)
                nc.tensor.matmul(out=fin[:, 0:w], lhsT=es2_t, rhs=sd2[:, 0:w],
                                 start=False, stop=False)
                nc.tensor.matmul(out=fin[:, 0:w], lhsT=w3_t, rhs=h2_t[:, 0:w],
                                 start=False, stop=True)
                if debug_dump:
                    nc.vector.tensor_copy(fin_sb[:, cols], fin[:, 0:w])
                nc.scalar.activation(out_sb[:, cols], fin[:, 0:w], SIGMOID)
            if c == NT - 2:
                # fire the first 3/4 of the output while the last chunk drains
                nc.sync.dma_start(out_d[:, 0:512 * (NT - 1)],
                                  out_sb[:, 0:512 * (NT - 1)])

        nc.sync.dma_start(out_d[:, 512 * (NT - 1):], out_sb[:, 512 * (NT - 1):])
        if debug_dump:
            nc.sync.dma_start(xdmp_d[:], x_t[:].bitcast(F32))
            nc.sync.dma_start(fdmp_d[:], fin_sb[:])
            for c in range(NT):
                nc.sync.dma_start(gdmp_d[:, c * CW:(c + 1) * CW], gbs[c][:])

    nc.compile()
    return nc


def _host_prep(sparse_feature, dense_feature, emb_table, W_dense, b_dense,
               w_first, b_first, W1, b1, W2, b2, W3, b3):
    """Build the augmented table, folded weights, and per-core in_maps."""
    f32 = np.float32
    emb_table = np.asarray(emb_table, dtype=f32)
    W_dense = np.asarray(W_dense, dtype=f32)      # [10, 7]
    b_dense = np.asarray(b_dense, dtype=f32)      # [10]
    w_first = np.asarray(w_first, dtype=f32)      # [S+7]
    b_first = np.asarray(b_first, dtype=f32)      # [1]
    W1 = np.asarray(W1, dtype=f32)                # [90, 128]
    b1 = np.asarray(b1, dtype=f32)                # [128]
    W2 = np.asarray(W2, dtype=f32)                # [128, 128]
    b2 = np.asarray(b2, dtype=f32)                # [128]
    W3 = np.asarray(W3, dtype=f32)                # [128, 1]
    b3 = np.asarray(b3, dtype=f32)                # [1]

    tab = np.zeros((S, RW), dtype=f32)
    tab[:, :EMB] = emb_table
    tab[:, EMB] = w_first[:S]

    w1s = np.zeros((KX, HID), dtype=f32)
    for f in range(F):
        w1s[f * RW:f * RW + EMB] = W1[f * EMB:(f + 1) * EMB]
    w1s[96:103] = W_dense.T @ W1[F * EMB:]               # [7,128]
    w1s[103] = b1 + b_dense @ W1[F * EMB:]

    # sdw: cols 0..9 -> s (sum of field embs + dense emb), 10..19 -> dense
    # emb alone
    sdw = np.zeros((KX, 20), dtype=f32)
    for f in range(F):
        for e in range(EMB):
            sdw[f * RW + e, e] = 1.0
    sdw[96:103, 0:10] = W_dense.T
    sdw[103, 0:10] = b_dense
    sdw[96:103, 10:20] = W_dense.T
    sdw[103, 10:20] = b_dense

    # a1: first-order linear term (wf rows + dense wfirst + biases)
    a1 = np.zeros((KX, 1), dtype=f32)
    for f in range(F):
        a1[f * RW + EMB] = 1.0
    a1[96:103, 0] = w_first[S:]
    a1[103] = b_first[0] + b3[0]

    # esq: -0.5 coefficients for the x^2 rows; es2: [+0.5 s^2, -0.5 demb^2]
    esq = np.zeros((96, 1), dtype=f32)
    for f in range(F):
        esq[f * RW:f * RW + EMB] = -0.5
    es2 = np.zeros((20, 1), dtype=f32)
    es2[0:10] = 0.5
    es2[10:20] = -0.5

    idx_g = (np.asarray(sparse_feature, dtype=np.int64)
             + OFFSETS[None, :]).astype(np.int32)         # [B, F]
    dense = np.asarray(dense_feature, dtype=f32)          # [B, 7]

    # cross-product table for fields 6 and 7
    t6 = tab[OFFSETS[6]:OFFSETS[6] + VOCABS[6]]           # [5000, 12]
    t7 = tab[OFFSETS[7]:OFFSETS[7] + VOCABS[7]]           # [1000, 12]
    t67 = np.empty((VOCABS[6] * VOCABS[7], 2 * RW), dtype=f32)
    t67[:, :RW] = np.repeat(t6, VOCABS[7], axis=0)
    t67[:, RW:] = np.tile(t7, (VOCABS[6], 1))
    sp = np.asarray(sparse_feature, dtype=np.int64)
    idx67 = (sp[:, 6] * VOCABS[7] + sp[:, 7]).astype(np.int32)  # [B]
    idx7 = np.concatenate([idx_g[:, :6], idx67[:, None]], axis=1)  # [B, 7]

    wpk = np.zeros((128, WPK), dtype=f32)
    wpk[:, 0:128] = np.eye(128, dtype=f32)
    wpk[0:KX, 128:256] = w1s
    wpk[:, 256:384] = W2
    wpk[:, 384] = b2
    wpk[0:KX, 385:405] = sdw
    wpk[0:KX, 405] = a1[:, 0]
    wpk[0:96, 406] = esq[:, 0]
    wpk[:, 407] = W3.reshape(HID)
    wpk[0:20, 408] = es2[:, 0]

    common = {"tab": tab, "t67": t67, "wpk": wpk}
    in_maps = []
    for c in range(N_CORES):
        lo, hi = c * BL, (c + 1) * BL
        lg = idx7[lo:hi].reshape(NBLK, 128, 7)
        idxs = np.ascontiguousarray(
            lg.transpose(1, 0, 2).reshape(128, NBLK * 7))  # [128, 112]
        dn8 = np.ones((8, BL), dtype=f32)
        dn8[:7] = dense[lo:hi].T
        in_maps.append(dict(common, idxs=idxs, dn8=dn8))
    return in_maps


def _get_program(debug_dump=False):
    key = ("nc", debug_dump)
    if key not in _cached:
        _cached[key] = _build_program(debug_dump)
    return _cached[key]


def run_on_device(in_maps, trace=False, debug_dump=False):
    """Run the SPMD program on 8 NeuronCores.  Returns (results, exec_time_ns)."""
    from concourse.bass_utils import run_bass_kernel_spmd

    nc = _get_program(debug_dump)
    res = run_bass_kernel_spmd(nc, in_maps, list(range(N_CORES)), trace=trace)
    return res.results, res.exec_time_ns


def kernel(**inputs):
    in_maps = _host_prep(**inputs)
    results, _ = run_on_device(in_maps, trace=False)
    out = np.concatenate([results[c]["out"].reshape(BL) for c in range(N_CORES)])
    return out.astype(np.float32)
